# revision 11
# baseline (speedup 1.0000x reference)
"""Decoupled-RoPE causal MHA on 8 Trainium2 NeuronCores (Bass/Tile).

Sharding: batch 4-way x head-group 2-way (8 shards). Core c handles batch
c//2 and heads (c%2)*8..(c%2)*8+8. qkv weights column-sharded per head, wo
row-sharded; per-core partial outputs are summed pairwise on the host
(the "all-reduce" after wo).

Device algorithm per core (all matmuls fp32r = full-rate fp32):
  Phase A: qT/kT = W_h @ x^T per head ([head_dim, S], RoPE applied with a
           host-permuted weight layout [nope|even|odd] + partition-swap via
           SBUF-SBUF DMA); V natural [S, head_dim] via 4-head-batched matmuls.
           Spilled to DRAM scratch.
  Phase B: per head, causal attention with scores^T = K_chunk^T Q orientation:
           exp on ACT (no max subtraction - scores are O(5)), additive causal
           mask pre-exp, PV matmuls accumulate ctx^T; softmax denominator via
           ones-matmul partition reduce + reciprocal + K=1 broadcast matmul.
  Phase C: out = ctx^T.T @ wo_local^T accumulated over the 8 local heads.
"""
import sys
import os

sys.path.insert(0, '/opt/trn_rl_repo')

import numpy as np

import concourse.bass as bass
import concourse.tile as tile
import concourse.mybir as mybir
from concourse.bass_utils import run_bass_kernel_spmd

f32 = mybir.dt.float32
f32r = mybir.dt.float32r

_wait_counter = [0]


def split_excess_waits(nc, max_waits: int = 1, verbose: bool = False):
    """This walrus build supports only one sync-wait slot per instruction;
    hoist excess waits into standalone EventSemaphore instructions."""
    n_split = 0
    for func in nc.m.functions:
        for bb in func.blocks:
            out = []
            changed = False
            for ins in bb.instructions:
                si = ins.sync_info
                if si is not None and si.on_wait and len(si.on_wait) > max_waits:
                    waits = list(si.on_wait)
                    for w in waits[:-max_waits]:
                        _wait_counter[0] += 1
                        ev = mybir.InstEventSemaphore(
                            name=f"I-waitsplit-{_wait_counter[0]}")
                        ev.engine = ins.engine
                        ev.sync_info = mybir.SyncInfo(on_wait=[w], on_update=[])
                        out.append(ev)
                    ins.sync_info = mybir.SyncInfo(
                        on_wait=waits[-max_waits:], on_update=list(si.on_update))
                    n_split += 1
                    changed = True
                out.append(ins)
            if changed:
                bb.instructions = out
    if verbose:
        print(f"wait_legalize: split {n_split} instructions")
    return n_split


B, S, D = 4, 2048, 2048
H_TOT, HD = 16, 128
HL = 8                      # heads per core
NCORES = 8
KO = D // 128               # 16 contraction chunks
SCALE = float(1.0 / np.sqrt(HD))
MASK_NEG = -30000.0


def build_program():
    nc = bass.Bass("TRN2", debug=False)

    xT = nc.dram_tensor("xT", [D, S], f32, kind="ExternalInput")
    w_qk = nc.dram_tensor("w_qk", [HL * 2, 128, KO, 128], f32, kind="ExternalInput")
    w_v4 = nc.dram_tensor("w_v4", [2, 128, KO, 512], f32, kind="ExternalInput")
    wo_t = nc.dram_tensor("wo_t", [128, HL, D], f32, kind="ExternalInput")
    cs_cos = nc.dram_tensor("cs_cos", [128, S], f32, kind="ExternalInput")
    cs_sin = nc.dram_tensor("cs_sin", [128, S], f32, kind="ExternalInput")
    masks = nc.dram_tensor("masks", [128, 4, 512], f32, kind="ExternalInput")
    ones_in = nc.dram_tensor("ones_in", [128, 128], f32, kind="ExternalInput")
    OUT = nc.dram_tensor("out", [S, D], f32, kind="ExternalOutput")

    xT_r = xT.ap().rearrange("(ko p) s -> p ko s", p=128)

    with tile.TileContext(nc) as tc:
        from contextlib import ExitStack
        with ExitStack() as ctx:
            constp = ctx.enter_context(tc.tile_pool(name="const", bufs=1))
            dramp = ctx.enter_context(tc.tile_pool(name="dram", bufs=1, space="DRAM"))

            ones_col = constp.tile([128, 1], f32r, name="ones_col")
            ones_row = constp.tile([1, 128], f32r, name="ones_row")
            nc.sync.dma_start(ones_col[:], ones_in.ap()[:, 0:1].bitcast(f32r))
            nc.sync.dma_start(ones_row[:], ones_in.ap()[0:1, :].bitcast(f32r))

            # DRAM scratch (per head)
            qT_d = [dramp.tile([128, S], f32r, name=f"qT_d{h}") for h in range(HL)]
            kT_d = [dramp.tile([128, S], f32r, name=f"kT_d{h}") for h in range(HL)]
            v_d = [dramp.tile([S, 128], f32r, name=f"v_d{h}") for h in range(HL)]

            # ---------------- Phase A: QKV ----------------
            with tc.tile_pool(name="xp", bufs=1) as xp:
                xT_sb = xp.tile([128, KO, S], f32r, name="xT")
                for ko in range(KO):
                    nc.sync.dma_start(xT_sb[:, ko, :], xT_r[:, ko, :].bitcast(f32r))

                with tc.tile_pool(name="acs", bufs=1) as acsp, \
                     tc.tile_pool(name="awv", bufs=1) as awvp, \
                     tc.tile_pool(name="aw", bufs=2) as awp, \
                     tc.tile_pool(name="ast", bufs=3) as astp, \
                     tc.tile_pool(name="asw", bufs=2) as aswp, \
                     tc.tile_pool(name="avst", bufs=2) as avstp, \
                     tc.tile_pool(name="apsA", bufs=6, space="PSUM") as apsA, \
                     tc.tile_pool(name="apsV", bufs=2, space="PSUM") as apsV:

                    cs_cos_sb = acsp.tile([128, S], f32r, name="cs_cos")
                    cs_sin_sb = acsp.tile([128, S], f32r, name="cs_sin")
                    nc.sync.dma_start(cs_cos_sb[:], cs_cos.ap().bitcast(f32r))
                    nc.sync.dma_start(cs_sin_sb[:], cs_sin.ap().bitcast(f32r))

                    # V for 2 groups of 4 heads, natural [S, d] layout
                    def emit_v_group(g):
                        wv_sb = awvp.tile([128, KO, 512], f32r, name="wv", tag="wv")
                        nc.sync.dma_start(wv_sb[:], w_v4.ap()[g].bitcast(f32r))
                        for sc in range(16):
                            ps = apsV.tile([128, 512], f32, name="v_ps", tag="v_ps")
                            for ko in range(KO):
                                nc.tensor.matmul(
                                    ps[:],
                                    xT_sb[:, ko, sc * 128:(sc + 1) * 128],
                                    wv_sb[:, ko, :],
                                    start=(ko == 0), stop=(ko == KO - 1),
                                )
                            vst = avstp.tile([128, 512], f32r, name="vst", tag="vst")
                            nc.vector.tensor_copy(vst[:], ps[:])
                            for j in range(4):
                                h = g * 4 + j
                                nc.sync.dma_start(
                                    v_d[h][sc * 128:(sc + 1) * 128, :],
                                    vst[:, j * 128:(j + 1) * 128],
                                )

                    # qT / kT per head, rope'd, transposed [d, S] layout
                    def emit_qk_head(h):
                        for part, dst in ((0, qT_d[h]), (1, kT_d[h])):
                            w_sb = awp.tile([128, KO, 128], f32r, name="wqk", tag="wqk")
                            nc.sync.dma_start(
                                w_sb[:], w_qk.ap()[h * 2 + part].bitcast(f32r))
                            for st in range(4):
                                sl = slice(st * 512, (st + 1) * 512)
                                ps = apsA.tile([128, 512], f32, name="qk_ps", tag="qk_ps")
                                for ko in range(KO):
                                    nc.tensor.matmul(
                                        ps[:], w_sb[:, ko, :], xT_sb[:, ko, sl],
                                        start=(ko == 0), stop=(ko == KO - 1),
                                    )
                                stg = astp.tile([128, 512], f32r, name="stg", tag="stg")
                                sw = aswp.tile([128, 512], f32r, name="sw", tag="sw")
                                nc.vector.tensor_copy(stg[:], ps[:])
                                nc.sync.dma_start(sw[64:96, :], stg[96:128, :])
                                nc.sync.dma_start(sw[96:128, :], stg[64:96, :])
                                nc.vector.tensor_mul(
                                    stg[64:128, :], stg[64:128, :], cs_cos_sb[64:128, sl])
                                nc.vector.tensor_mul(
                                    sw[64:128, :], sw[64:128, :], cs_sin_sb[64:128, sl])
                                nc.vector.tensor_add(
                                    stg[64:128, :], stg[64:128, :], sw[64:128, :])
                                nc.sync.dma_start(dst[:, sl], stg[:])

                    emit_v_group(0)
                    for h in range(4):
                        emit_qk_head(h)
                    emit_v_group(1)
                    for h in range(4, HL):
                        emit_qk_head(h)

            # ---------------- Phase B: attention ----------------
            with tc.tile_pool(name="ctxall", bufs=1) as ctxp:
                ctx_all = ctxp.tile([128, HL, S], f32r, name="ctx_all")

                with tc.tile_pool(name="bmask", bufs=1) as bmaskp, \
                     tc.tile_pool(name="bqk", bufs=2) as bqkp, \
                     tc.tile_pool(name="bv", bufs=2) as bvp, \
                     tc.tile_pool(name="bp", bufs=3) as bpp, \
                     tc.tile_pool(name="bacc", bufs=2) as baccp, \
                     tc.tile_pool(name="bden", bufs=2) as bdenp, \
                     tc.tile_pool(name="bpsS", bufs=2, space="PSUM") as bpsS, \
                     tc.tile_pool(name="bpsC", bufs=2, space="PSUM") as bpsC, \
                     tc.tile_pool(name="bpsD", bufs=1, space="PSUM") as bpsD, \
                     tc.tile_pool(name="bpsB", bufs=1, space="PSUM") as bpsB:

                    masks_sb = bmaskp.tile([128, 4, 512], f32, name="masks")
                    nc.sync.dma_start(masks_sb[:], masks.ap())

                    for h in range(HL):
                        qT_sb = bqkp.tile([128, S], f32r, name="qT", tag="qT")
                        kT_sb = bqkp.tile([128, S], f32r, name="kT", tag="kT")
                        v_sb = bvp.tile([128, 16, 128], f32r, name="v", tag="v")
                        nc.sync.dma_start(qT_sb[:], qT_d[h][:])
                        nc.sync.dma_start(kT_sb[:], kT_d[h][:])
                        nc.sync.dma_start(
                            v_sb[:], v_d[h][:].rearrange("(sc p) d -> p sc d", p=128))

                        for qc in range(4):
                            q0 = qc * 512
                            qsl = slice(q0, q0 + 512)
                            ngrp = 2 * (qc + 1)
                            ctx_ps = bpsC.tile([128, 512], f32, name="ctx_ps", tag="ctx_ps")
                            acc = baccp.tile([128, 512], f32r, name="acc", tag="acc")
                            for kg in range(ngrp):
                                sps = bpsS.tile([128, 2, 512], f32, name="sps", tag="sps")
                                for j in range(2):
                                    kc = 2 * kg + j
                                    nc.tensor.matmul(
                                        sps[:, j, :],
                                        kT_sb[:, kc * 128:(kc + 1) * 128],
                                        qT_sb[:, qsl],
                                        start=True, stop=True,
                                    )
                                if kg >= ngrp - 2:
                                    jj = kg - (ngrp - 2)
                                    nc.vector.tensor_add(
                                        sps[:], sps[:],
                                        masks_sb[:, 2 * jj:2 * jj + 2, :])
                                p_sb = bpp.tile([128, 2, 512], f32r, name="p", tag="p")
                                nc.scalar.activation(
                                    p_sb[:], sps[:],
                                    mybir.ActivationFunctionType.Exp, scale=SCALE)
                                for j in range(2):
                                    kc = 2 * kg + j
                                    nc.tensor.matmul(
                                        ctx_ps[:],
                                        v_sb[:, kc, :],
                                        p_sb[:, j, :],
                                        start=(kg == 0 and j == 0),
                                        stop=(kg == ngrp - 1 and j == 1),
                                    )
                                if kg == 0:
                                    nc.vector.tensor_add(
                                        acc[:], p_sb[:, 0, :], p_sb[:, 1, :])
                                else:
                                    nc.vector.tensor_add(acc[:], acc[:], p_sb[:, 0, :])
                                    nc.vector.tensor_add(acc[:], acc[:], p_sb[:, 1, :])
                            den_ps = bpsD.tile([1, 512], f32, name="den_ps", tag="den_ps")
                            nc.tensor.matmul(den_ps[:], ones_col[:], acc[:],
                                             start=True, stop=True)
                            recip = bdenp.tile([1, 512], f32r, name="recip", tag="recip")
                            with nc.allow_low_precision(reason="f32r recip row"):
                                nc.vector.reciprocal(recip[:], den_ps[:])
                            bc_ps = bpsB.tile([128, 512], f32, name="bc_ps", tag="bc_ps")
                            nc.tensor.matmul(bc_ps[:], ones_row[:], recip[:],
                                             start=True, stop=True)
                            bc_sb = bdenp.tile([128, 512], f32, name="bc_sb", tag="bc_sb")
                            nc.vector.tensor_copy(bc_sb[:], bc_ps[:])
                            nc.vector.tensor_mul(
                                ctx_all[:, h, qsl], ctx_ps[:], bc_sb[:])

                # ---------------- Phase C: wo ----------------
                with tc.tile_pool(name="cwo", bufs=1) as cwop, \
                     tc.tile_pool(name="cout", bufs=4) as coutp, \
                     tc.tile_pool(name="cps", bufs=6, space="PSUM") as cps:
                    wo_sb = cwop.tile([128, HL, D], f32r, name="wo")
                    nc.sync.dma_start(wo_sb[:], wo_t.ap().bitcast(f32r))
                    for sc in range(16):
                        ssl = slice(sc * 128, (sc + 1) * 128)
                        for et in range(4):
                            esl = slice(et * 512, (et + 1) * 512)
                            ops = cps.tile([128, 512], f32, name="o_ps", tag="o_ps")
                            for fo in range(HL):
                                nc.tensor.matmul(
                                    ops[:],
                                    ctx_all[:, fo, ssl],
                                    wo_sb[:, fo, esl],
                                    start=(fo == 0), stop=(fo == HL - 1),
                                )
                            osb = coutp.tile([128, 512], f32, name="o_sb", tag="o_sb")
                            nc.vector.tensor_copy(osb[:], ops[:])
                            nc.sync.dma_start(OUT.ap()[ssl, esl], osb[:])

    split_excess_waits(nc, verbose=True)
    return nc


def prepare_inputs(x, qkv_w, wo, cos_cached, sin_cached):
    x = np.ascontiguousarray(np.asarray(x, dtype=np.float32))
    qkv_w = np.asarray(qkv_w, dtype=np.float32)
    wo = np.asarray(wo, dtype=np.float32)
    cos = np.asarray(cos_cached, dtype=np.float32)[:S]
    sin = np.asarray(sin_cached, dtype=np.float32)[:S]

    Wq, Wk, Wv = qkv_w[0:D], qkv_w[D:2 * D], qkv_w[2 * D:3 * D]
    perm = np.concatenate(
        [np.arange(64), 64 + 2 * np.arange(32), 65 + 2 * np.arange(32)])

    cosT, sinT = cos.T, sin.T                      # [32, S]
    cs_cos = np.zeros((128, S), dtype=np.float32)
    cs_sin = np.zeros((128, S), dtype=np.float32)
    cs_cos[64:96] = cosT
    cs_cos[96:128] = cosT
    cs_sin[64:96] = -sinT
    cs_sin[96:128] = sinT

    kk = np.arange(128)[:, None, None]
    rr = np.arange(4)[None, :, None]
    qq = np.arange(512)[None, None, :]
    masks = np.where(kk + rr * 128 <= qq, 0.0, MASK_NEG).astype(np.float32)

    def w_lhsT(wm):     # [128 rows, D] -> [128p, KO, 128m]
        return np.ascontiguousarray(
            wm.T.reshape(KO, 128, wm.shape[0]).transpose(1, 0, 2))

    in_maps = []
    for c in range(NCORES):
        b, g = c // 2, c % 2
        heads = range(g * HL, g * HL + HL)
        xT = np.ascontiguousarray(x[b].T)
        w_qk = np.empty((HL * 2, 128, KO, 128), dtype=np.float32)
        for i, h in enumerate(heads):
            w_qk[2 * i] = w_lhsT(Wq[h * HD:(h + 1) * HD][perm])
            w_qk[2 * i + 1] = w_lhsT(Wk[h * HD:(h + 1) * HD][perm])
        w_v4 = np.empty((2, 128, KO, 512), dtype=np.float32)
        for grp in range(2):
            hs = list(heads)[grp * 4:(grp + 1) * 4]
            wv = np.concatenate([Wv[h * HD:(h + 1) * HD] for h in hs], axis=0)
            w_v4[grp] = wv.T.reshape(KO, 128, 512).transpose(1, 0, 2)
        wo_t = np.ascontiguousarray(
            np.stack([wo[:, h * HD:(h + 1) * HD].T for h in heads], 0)
            .transpose(1, 0, 2))                    # [128, HL, D]
        in_maps.append({
            "xT": xT, "w_qk": w_qk, "w_v4": w_v4, "wo_t": wo_t,
            "cs_cos": cs_cos, "cs_sin": cs_sin, "masks": masks,
            "ones_in": np.ones((128, 128), dtype=np.float32),
        })
    return in_maps


_NC = None


def _get_program():
    global _NC
    if _NC is None:
        _NC = build_program()
    return _NC


def run(inputs, trace=False, trace_cores=None):
    nc = _get_program()
    in_maps = prepare_inputs(**inputs)
    res = run_bass_kernel_spmd(
        nc, in_maps, core_ids=list(range(NCORES)),
        trace=trace, trace_cores=trace_cores)
    outs = [r["out"] for r in res.results]
    full = np.empty((B, S, D), dtype=np.float32)
    for b in range(B):
        full[b] = outs[2 * b] + outs[2 * b + 1]
    return full, res


def kernel(**inputs) -> np.ndarray:
    out, _ = run(inputs, trace=False)
    return out


# revision 23
# speedup vs baseline: 80.3141x; 80.3141x over previous
"""Decoupled-RoPE causal MHA on 8 Trainium2 NeuronCores (Bass/Tile).

Sharding: batch 4-way x head-group 2-way (8 shards). Core c handles batch
c//2 and heads (c%2)*8..(c%2)*8+8. qkv weights column-sharded per head, wo
row-sharded; per-core partial outputs are summed pairwise on the host
(the "all-reduce" after wo).

Device algorithm per core (all matmuls fp32r = full-rate fp32):
  Phase A: qT/kT = W_h @ x^T per head ([head_dim, S], RoPE applied with a
           host-permuted weight layout [nope|even|odd] + partition-swap via
           SBUF-SBUF DMA); V natural [S, head_dim] via 4-head-batched matmuls.
           Spilled to DRAM scratch.
  Phase B: per head, causal attention with scores^T = K_chunk^T Q orientation:
           exp on ACT (no max subtraction - scores are O(5)), multiplicative
           0/1 causal mask after exp (SBUF-only DVE), PV matmuls accumulate
           ctx^T; softmax denominator accumulated with ones-column matmuls in
           PSUM, then reciprocal + K=1 broadcast matmul for the row-wise scale.
  Phase C: out = ctx^T.T @ wo_local^T accumulated over the 8 local heads.
"""
import sys
import os

sys.path.insert(0, '/opt/trn_rl_repo')

import numpy as np

import concourse.bass as bass
import concourse.tile as tile
import concourse.mybir as mybir
from concourse.bass_utils import run_bass_kernel_spmd

f32 = mybir.dt.float32
f32r = mybir.dt.float32r

_wait_counter = [0]


def split_excess_waits(nc, max_waits: int = 1, verbose: bool = False):
    """This walrus build supports only one sync-wait slot per instruction;
    hoist excess waits into standalone EventSemaphore instructions."""
    n_split = 0
    for func in nc.m.functions:
        for bb in func.blocks:
            out = []
            changed = False
            for ins in bb.instructions:
                si = ins.sync_info
                if si is not None and si.on_wait and len(si.on_wait) > max_waits:
                    waits = list(si.on_wait)
                    for w in waits[:-max_waits]:
                        _wait_counter[0] += 1
                        ev = mybir.InstEventSemaphore(
                            name=f"I-waitsplit-{_wait_counter[0]}")
                        ev.engine = ins.engine
                        ev.sync_info = mybir.SyncInfo(on_wait=[w], on_update=[])
                        out.append(ev)
                    ins.sync_info = mybir.SyncInfo(
                        on_wait=waits[-max_waits:], on_update=list(si.on_update))
                    n_split += 1
                    changed = True
                out.append(ins)
            if changed:
                bb.instructions = out
    if verbose:
        print(f"wait_legalize: split {n_split} instructions")
    return n_split


B, S, D = 4, 2048, 2048
H_TOT, HD = 16, 128
HL = 8                      # heads per core
NCORES = 8
KO = D // 128               # 16 contraction chunks
SCALE = float(1.0 / np.sqrt(HD))


def build_program(phases="ABC"):
    nc = bass.Bass("TRN2", debug=False)

    xT = nc.dram_tensor("xT", [D, S], f32, kind="ExternalInput")
    w_qk = nc.dram_tensor("w_qk", [HL * 2, 128, KO, 128], f32, kind="ExternalInput")
    w_v4 = nc.dram_tensor("w_v4", [2, 128, KO, 512], f32, kind="ExternalInput")
    wo_t = nc.dram_tensor("wo_t", [128, HL, D], f32, kind="ExternalInput")
    cs_cos = nc.dram_tensor("cs_cos", [128, S], f32, kind="ExternalInput")
    cs_sin = nc.dram_tensor("cs_sin", [128, S], f32, kind="ExternalInput")
    masks = nc.dram_tensor("masks", [128, 4, 512], f32, kind="ExternalInput")
    ones_in = nc.dram_tensor("ones_in", [128, 128], f32, kind="ExternalInput")
    perm_sw = nc.dram_tensor("perm_sw", [128, 64], f32, kind="ExternalInput")
    OUT = nc.dram_tensor("out", [S, D], f32, kind="ExternalOutput")

    xT_r = xT.ap().rearrange("(ko p) s -> p ko s", p=128)

    with tile.TileContext(nc) as tc:
        from contextlib import ExitStack
        with ExitStack() as ctx:
            constp = ctx.enter_context(tc.tile_pool(name="const", bufs=1))
            dramp = ctx.enter_context(tc.tile_pool(name="dram", bufs=1, space="DRAM"))

            perm_sb = constp.tile([128, 64], f32r, name="perm_sw")
            nc.sync.dma_start(perm_sb[:], perm_sw.ap().bitcast(f32r))
            ones_col = constp.tile([128, 1], f32r, name="ones_col")
            ones_row = constp.tile([1, 128], f32r, name="ones_row")
            nc.sync.dma_start(ones_col[:], ones_in.ap()[:, 0:1].bitcast(f32r))
            nc.sync.dma_start(ones_row[:], ones_in.ap()[0:1, :].bitcast(f32r))

            # DRAM scratch (per head)
            qT_d = [dramp.tile([128, S], f32r, name=f"qT_d{h}") for h in range(HL)]
            kT_d = [dramp.tile([128, S], f32r, name=f"kT_d{h}") for h in range(HL)]
            v_d = [dramp.tile([S, 128], f32r, name=f"v_d{h}") for h in range(HL)]

            # ---------------- Phase A: QKV ----------------
            with tc.tile_pool(name="xp", bufs=1) as xp:
                xT_sb = xp.tile([128, KO, S], f32r, name="xT")
                for ko in range(KO):
                    nc.sync.dma_start(xT_sb[:, ko, :], xT_r[:, ko, :].bitcast(f32r))

                with tc.tile_pool(name="acs", bufs=1) as acsp, \
                     tc.tile_pool(name="awv", bufs=1) as awvp, \
                     tc.tile_pool(name="aw", bufs=2) as awp, \
                     tc.tile_pool(name="ast", bufs=3) as astp, \
                     tc.tile_pool(name="asw", bufs=2) as aswp, \
                     tc.tile_pool(name="avst", bufs=2) as avstp, \
                     tc.tile_pool(name="apsA", bufs=6, space="PSUM") as apsA, \
                     tc.tile_pool(name="apsV", bufs=2, space="PSUM") as apsV:

                    cs_cos_sb = acsp.tile([128, S], f32r, name="cs_cos")
                    cs_sin_sb = acsp.tile([128, S], f32r, name="cs_sin")
                    nc.sync.dma_start(cs_cos_sb[:], cs_cos.ap().bitcast(f32r))
                    nc.sync.dma_start(cs_sin_sb[:], cs_sin.ap().bitcast(f32r))

                    # V for 2 groups of 4 heads, natural [S, d] layout
                    def emit_v_group(g):
                        wv_sb = awvp.tile([128, KO, 512], f32r, name="wv", tag="wv")
                        nc.sync.dma_start(wv_sb[:], w_v4.ap()[g].bitcast(f32r))
                        for sc in range(16):
                            ps = apsV.tile([128, 512], f32, name="v_ps", tag="v_ps")
                            for ko in range(KO):
                                nc.tensor.matmul(
                                    ps[:],
                                    xT_sb[:, ko, sc * 128:(sc + 1) * 128],
                                    wv_sb[:, ko, :],
                                    start=(ko == 0), stop=(ko == KO - 1),
                                )
                            vst = avstp.tile([128, 512], f32r, name="vst", tag="vst")
                            nc.scalar.copy(vst[:], ps[:])
                            for j in range(4):
                                h = g * 4 + j
                                nc.sync.dma_start(
                                    v_d[h][sc * 128:(sc + 1) * 128, :],
                                    vst[:, j * 128:(j + 1) * 128],
                                )

                    # qT / kT per head, rope'd, transposed [d, S] layout
                    def emit_qk_head(h):
                        for part, dst in ((0, qT_d[h]), (1, kT_d[h])):
                            w_sb = awp.tile([128, KO, 128], f32r, name="wqk", tag="wqk")
                            nc.sync.dma_start(
                                w_sb[:], w_qk.ap()[h * 2 + part].bitcast(f32r))
                            for st in range(4):
                                sl = slice(st * 512, (st + 1) * 512)
                                ps = apsA.tile([128, 512], f32, name="qk_ps", tag="qk_ps")
                                for ko in range(KO):
                                    nc.tensor.matmul(
                                        ps[:], w_sb[:, ko, :], xT_sb[:, ko, sl],
                                        start=(ko == 0), stop=(ko == KO - 1),
                                    )
                                stg = astp.tile([128, 512], f32r, name="stg", tag="stg")
                                sw = aswp.tile([128, 512], f32r, name="sw", tag="sw")
                                nc.scalar.copy(stg[:], ps[:])
                                nc.sync.dma_start(sw[64:96, :], stg[96:128, :])
                                nc.sync.dma_start(sw[96:128, :], stg[64:96, :])
                                nc.vector.tensor_mul(
                                    stg[64:128, :], stg[64:128, :], cs_cos_sb[64:128, sl])
                                nc.vector.tensor_mul(
                                    sw[64:128, :], sw[64:128, :], cs_sin_sb[64:128, sl])
                                nc.vector.tensor_add(
                                    stg[64:128, :], stg[64:128, :], sw[64:128, :])
                                nc.sync.dma_start(dst[:, sl], stg[:])

                    only_v = "v" in phases
                    only_q = "q" in phases
                    if not only_q:
                        emit_v_group(0)
                    if not only_v:
                        for h in range(4):
                            emit_qk_head(h)
                    if not only_q:
                        emit_v_group(1)
                    if not only_v:
                        for h in range(4, HL):
                            emit_qk_head(h)

            # ---------------- Phase B: attention ----------------
            if "B" not in phases:
                return_early = True
            with tc.tile_pool(name="ctxall", bufs=1) as ctxp, \
                 tc.tile_pool(name="cwo", bufs=1) as cwop:
                ctx_all = ctxp.tile([128, HL, S], f32r, name="ctx_all")
                wo_sb = cwop.tile([128, HL, D], f32r, name="wo")
                nc.sync.dma_start(wo_sb[:], wo_t.ap().bitcast(f32r))

                with tc.tile_pool(name="bmask", bufs=1) as bmaskp, \
                     tc.tile_pool(name="bqk", bufs=2) as bqkp, \
                     tc.tile_pool(name="bv", bufs=2) as bvp, \
                     tc.tile_pool(name="bp", bufs=3) as bpp, \
                     tc.tile_pool(name="bden", bufs=2) as bdenp, \
                     tc.tile_pool(name="bpsS", bufs=2, space="PSUM") as bpsS, \
                     tc.tile_pool(name="bpsC", bufs=2, space="PSUM") as bpsC, \
                     tc.tile_pool(name="bpsD", bufs=1, space="PSUM") as bpsD, \
                     tc.tile_pool(name="bpsB", bufs=1, space="PSUM") as bpsB:

                    masks_sb = bmaskp.tile([128, 4, 512], f32, name="masks")
                    nc.sync.dma_start(masks_sb[:], masks.ap())

                    for h in (range(HL) if "B" in phases else range(0)):
                        qT_sb = bqkp.tile([128, S], f32r, name="qT", tag="qT")
                        kT_sb = bqkp.tile([128, S], f32r, name="kT", tag="kT")
                        v_sb = bvp.tile([128, 16, 128], f32r, name="v", tag="v")
                        nc.sync.dma_start(qT_sb[:], qT_d[h][:])
                        nc.sync.dma_start(kT_sb[:], kT_d[h][:])
                        nc.sync.dma_start(
                            v_sb[:], v_d[h][:].rearrange("(sc p) d -> p sc d", p=128))

                        for qc in range(4):
                            q0 = qc * 512
                            qsl = slice(q0, q0 + 512)
                            ngrp = 2 * (qc + 1)
                            ctx_ps = bpsC.tile([128, 512], f32, name="ctx_ps", tag="ctx_ps")
                            den_ps = bpsD.tile([1, 512], f32, name="den_ps", tag="den_ps")
                            for kg in range(ngrp):
                                sps = bpsS.tile([128, 2, 512], f32, name="sps", tag="sps")
                                for j in range(2):
                                    kc = 2 * kg + j
                                    nc.tensor.matmul(
                                        sps[:, j, :],
                                        kT_sb[:, kc * 128:(kc + 1) * 128],
                                        qT_sb[:, qsl],
                                        start=True, stop=True,
                                    )
                                p_sb = bpp.tile([128, 2, 512], f32r, name="p", tag="p")
                                nc.scalar.activation(
                                    p_sb[:], sps[:],
                                    mybir.ActivationFunctionType.Exp, scale=SCALE)
                                if kg >= ngrp - 2:
                                    jj = kg - (ngrp - 2)
                                    nc.vector.tensor_mul(
                                        p_sb[:], p_sb[:],
                                        masks_sb[:, 2 * jj:2 * jj + 2, :])
                                for j in range(2):
                                    kc = 2 * kg + j
                                    nc.tensor.matmul(
                                        ctx_ps[:],
                                        v_sb[:, kc, :],
                                        p_sb[:, j, :],
                                        start=(kg == 0 and j == 0),
                                        stop=(kg == ngrp - 1 and j == 1),
                                    )
                                for j in range(2):
                                    nc.tensor.matmul(
                                        den_ps[:], ones_col[:], p_sb[:, j, :],
                                        start=(kg == 0 and j == 0),
                                        stop=(kg == ngrp - 1 and j == 1),
                                    )
                            recip = bdenp.tile([1, 512], f32r, name="recip", tag="recip")
                            with nc.allow_low_precision(reason="f32r recip row"):
                                nc.vector.reciprocal(recip[:], den_ps[:])
                            bc_ps = bpsB.tile([128, 512], f32, name="bc_ps", tag="bc_ps")
                            nc.tensor.matmul(bc_ps[:], ones_row[:], recip[:],
                                             start=True, stop=True)
                            bc_sb = bdenp.tile([128, 512], f32, name="bc_sb", tag="bc_sb")
                            nc.vector.tensor_copy(bc_sb[:], bc_ps[:])
                            nc.vector.tensor_mul(
                                ctx_all[:, h, qsl], ctx_ps[:], bc_sb[:])

                # ---------------- Phase C: wo ----------------
                with tc.tile_pool(name="cout", bufs=4) as coutp, \
                     tc.tile_pool(name="cps", bufs=6, space="PSUM") as cps:
                    for sc in (range(16) if "C" in phases else range(0)):
                        ssl = slice(sc * 128, (sc + 1) * 128)
                        for et in range(4):
                            esl = slice(et * 512, (et + 1) * 512)
                            ops = cps.tile([128, 512], f32, name="o_ps", tag="o_ps")
                            for fo in range(HL):
                                nc.tensor.matmul(
                                    ops[:],
                                    ctx_all[:, fo, ssl],
                                    wo_sb[:, fo, esl],
                                    start=(fo == 0), stop=(fo == HL - 1),
                                )
                            osb = coutp.tile([128, 512], f32, name="o_sb", tag="o_sb")
                            nc.scalar.copy(osb[:], ops[:])
                            nc.sync.dma_start(OUT.ap()[ssl, esl], osb[:])

    split_excess_waits(nc, verbose=True)
    return nc


def prepare_inputs(x, qkv_w, wo, cos_cached, sin_cached):
    x = np.ascontiguousarray(np.asarray(x, dtype=np.float32))
    qkv_w = np.asarray(qkv_w, dtype=np.float32)
    wo = np.asarray(wo, dtype=np.float32)
    cos = np.asarray(cos_cached, dtype=np.float32)[:S]
    sin = np.asarray(sin_cached, dtype=np.float32)[:S]

    Wq, Wk, Wv = qkv_w[0:D], qkv_w[D:2 * D], qkv_w[2 * D:3 * D]
    perm = np.concatenate(
        [np.arange(64), 64 + 2 * np.arange(32), 65 + 2 * np.arange(32)])

    cosT, sinT = cos.T, sin.T                      # [32, S]
    cs_cos = np.zeros((128, S), dtype=np.float32)
    cs_sin = np.zeros((128, S), dtype=np.float32)
    cs_cos[64:96] = cosT
    cs_cos[96:128] = cosT
    cs_sin[64:96] = -sinT
    cs_sin[96:128] = sinT

    kk = np.arange(128)[:, None, None]
    rr = np.arange(4)[None, :, None]
    qq = np.arange(512)[None, None, :]
    masks = (kk + rr * 128 <= qq).astype(np.float32)

    perm_sw_np = np.zeros((128, 64), dtype=np.float32)
    for i in range(32):
        perm_sw_np[96 + i, i] = 1.0
        perm_sw_np[64 + i, 32 + i] = 1.0

    def w_lhsT(wm):     # [128 rows, D] -> [128p, KO, 128m]
        return np.ascontiguousarray(
            wm.T.reshape(KO, 128, wm.shape[0]).transpose(1, 0, 2))

    in_maps = []
    for c in range(NCORES):
        b, g = c // 2, c % 2
        heads = range(g * HL, g * HL + HL)
        xT = np.ascontiguousarray(x[b].T)
        w_qk = np.empty((HL * 2, 128, KO, 128), dtype=np.float32)
        for i, h in enumerate(heads):
            w_qk[2 * i] = w_lhsT(Wq[h * HD:(h + 1) * HD][perm])
            w_qk[2 * i + 1] = w_lhsT(Wk[h * HD:(h + 1) * HD][perm])
        w_v4 = np.empty((2, 128, KO, 512), dtype=np.float32)
        for grp in range(2):
            hs = list(heads)[grp * 4:(grp + 1) * 4]
            wv = np.concatenate([Wv[h * HD:(h + 1) * HD] for h in hs], axis=0)
            w_v4[grp] = wv.T.reshape(KO, 128, 512).transpose(1, 0, 2)
        wo_t = np.ascontiguousarray(
            np.stack([wo[:, h * HD:(h + 1) * HD].T for h in heads], 0)
            .transpose(1, 0, 2))                    # [128, HL, D]
        in_maps.append({
            "xT": xT, "w_qk": w_qk, "w_v4": w_v4, "wo_t": wo_t,
            "cs_cos": cs_cos, "cs_sin": cs_sin, "masks": masks,
            "ones_in": np.ones((128, 128), dtype=np.float32),
            "perm_sw": perm_sw_np,
        })
    return in_maps


_NC = None


def _get_program():
    global _NC
    if _NC is None:
        _NC = build_program()
    return _NC


def run(inputs, trace=False, trace_cores=None):
    nc = _get_program()
    in_maps = prepare_inputs(**inputs)
    res = run_bass_kernel_spmd(
        nc, in_maps, core_ids=list(range(NCORES)),
        trace=trace, trace_cores=trace_cores)
    outs = [r["out"] for r in res.results]
    full = np.empty((B, S, D), dtype=np.float32)
    for b in range(B):
        full[b] = outs[2 * b] + outs[2 * b + 1]
    return full, res


def kernel(**inputs) -> np.ndarray:
    out, _ = run(inputs, trace=False)
    return out
